# revision 18
# baseline (speedup 1.0000x reference)
"""CRF negative-log-likelihood loss kernel for Trainium2 (8 NeuronCores).

Problem: B=256, S=2048, T=64 CRF loss (torchcrf-style), mask all-ones.

Strategy (v2)
-------------
Data-parallel over batch: each of the 8 cores gets 32 batch rows.

Denominator (log-partition): forward/backward meet-in-the-middle.  The
forward chain  E_p = X_p * (W^T E_{p-1})  and the backward chain
C_s = X_s * (W C_{s+1})  (exp domain, X_s = exp(em_s - c0), W =
exp(trans)) advance together: one 128x128 block-diagonal matmul (top
block W, bottom block W^T as lhsT, bf16) + one [128,32] DVE multiply
per round.  1023 rounds instead of 2047.  Z = E_{S/2-1} . (W C_{S/2}).

Renormalization (v2, off the critical path): every RN rounds the
reciprocal of one PSUM row per direction (rows 0 / 64 of p — a mass
proxy, exact value irrelevant) is taken in bf16, logged into a ring
(racc output, host adds sum(log) back), broadcast to 128 rows via a
tiny PE matmul two rounds later, and applied DELTA rounds later by
pre-scaling that round's x-slice on the DVE.  Nothing on the chain's
PE->DVE->PE path waits for ACT/Pool.

Numerator (v2):
  trans part: host computes the pair-count matrix cnt[t,t'] from tags
    (index data only); device does one fused multiply-accumulate with
    trans.  -> 1 DVE instruction.
  emission part: per seq-chunk, one-hot oh[s,t,b] = (iota_tb == tag
    broadcast) on DVE (bf16, 2x mode), then fused (oh * em) with
    per-partition accumulation into asum columns.  Split into 4 slabs
    per chunk so no DVE instruction exceeds ~300ns (no chain stalls).
  start/end transitions are folded into em rows s=0 / s=S-1 on the
  host (also makes X_0 / X_{S-1} the correct chain initializers).

Device outputs are raw (z, renorm ring, asum/tsum partials); the final
logs/sums are host-side scalar work on tiny tensors.

Per-core outputs: z[1,32] f32, racc[2,NRN,32] bf16, asum[128,64] f32,
tsum[64,1] f32.  Host: den_b = ln(z_b)+S*c0-sum_k ln(racc[:,k,b]);
loss = -(sum(asum)+sum(tsum) - sum(den)) / B.
"""

import contextlib

import numpy as np
import ml_dtypes

F32_NP = np.float32
BF16_NP = ml_dtypes.bfloat16

B, S, T = 256, 2048, 64
NCORES = 8
BSH = B // NCORES  # 32
CHUNK = 128
C0 = 4.8204  # ~ ln(64 * e^0.5 * sinh(1)) : expected per-step log growth
RN = 128     # renorm every RN rounds (drift stays << f32 range)
DELTA = 8    # rounds between logging a renorm scale and applying it

_NC_CACHE = {}


def build(n_chunks=16, bsh=BSH, nrep=1, rn=RN, no_num=False, fake_x=False,
          pround_bufs=4, spool_bufs=6, num_slabs=4, chain_on_pool=False,
          num_bcast=True, num_on_pool=False, num_pool_ts=False,
          pe_warm=0):
    """Build + compile the per-core Bass module. n_chunks*128 = seq len."""
    import concourse.bacc as bacc
    import concourse.mybir as mybir
    import concourse.tile as tile

    F32 = mybir.dt.float32
    BF16 = mybir.dt.bfloat16
    AF = mybir.ActivationFunctionType
    ALU = mybir.AluOpType

    s_len = n_chunks * CHUNK
    half = n_chunks // 2
    assert half * 2 == n_chunks and half >= 1
    n_rounds = half * CHUNK - 1
    # renorm rounds: r = rn, 2*rn, ... with r + DELTA <= n_rounds
    ren_rounds = [r for r in range(rn, n_rounds + 1, rn) if r + DELTA <= n_rounds]
    n_ren = max(1, len(ren_rounds))
    slab_b = bsh // num_slabs  # batch-extent of one numerator slab

    nc = bacc.Bacc("TRN2", target_bir_lowering=False, debug=False,
                   num_devices=NCORES)

    em_x_d = nc.dram_tensor("emx", [half, 128, 128, bsh], BF16,
                            kind="ExternalInput")
    em_m_d = nc.dram_tensor("emm", [n_chunks, 128, bsh, T], BF16,
                            kind="ExternalInput")
    tags_d = nc.dram_tensor("tagst", [128, n_chunks * bsh], BF16,
                            kind="ExternalInput")
    tagsf_d = nc.dram_tensor("tagsf", [128, n_chunks * bsh], F32,
                            kind="ExternalInput")
    cnt_d = nc.dram_tensor("cnt", [T, T], F32, kind="ExternalInput")
    trans_d = nc.dram_tensor("trans", [T, T], F32, kind="ExternalInput")
    bsel_d = nc.dram_tensor("bsel", [2, 128], BF16, kind="ExternalInput")
    bones_d = nc.dram_tensor("bones", [128, 2], BF16, kind="ExternalInput")
    iota_d = nc.dram_tensor("iotat", [128, bsh * T], BF16,
                            kind="ExternalInput")
    transt_d = nc.dram_tensor("transt", [T, T], F32, kind="ExternalInput")
    z_d = nc.dram_tensor("z", [1, bsh], F32, kind="ExternalOutput")
    racc_d = nc.dram_tensor("racc", [2, n_ren * bsh], BF16,
                            kind="ExternalOutput")
    asum_d = nc.dram_tensor("asum", [128, n_chunks * num_slabs], F32,
                            kind="ExternalOutput")
    tsum_d = nc.dram_tensor("tsum", [T, 1], F32, kind="ExternalOutput")

    ew = nc.gpsimd if chain_on_pool else nc.vector
    nv = nc.gpsimd if num_on_pool else nc.vector

    with tile.TileContext(nc) as tc, nc.allow_low_precision(
            reason="bf16 state/weights validated against f64 reference"):
        with (
            tc.tile_pool(name="consts", bufs=1) as consts,
            tc.tile_pool(name="xchunk", bufs=3) as xpool,
            tc.tile_pool(name="xraw", bufs=3) as xrawpool,
            tc.tile_pool(name="emt", bufs=6) as empool,
            tc.tile_pool(name="numscr", bufs=4) as numscr,
            tc.tile_pool(name="state", bufs=spool_bufs) as spool,
            tc.tile_pool(name="small", bufs=4) as smallpool,
            tc.tile_pool(name="pround", bufs=pround_bufs,
                         space="PSUM") as pround,
            tc.tile_pool(name="prbc", bufs=2, space="PSUM") as prbc,
            tc.tile_pool(name="pmass", bufs=1, space="PSUM") as pmass,
            tc.tile_pool(name="pdummy", bufs=1, space="PSUM") as pdummy,
        ):
            rep_ctx = (tc.For_i(0, nrep, 1) if nrep > 1
                       else contextlib.nullcontext())
            with rep_ctx:
                # ---------------- constants / setup ----------------
                transt_sb = consts.tile([T, T], F32, tag="transt")
                nc.sync.dma_start(transt_sb[:], transt_d.ap())
                iota_bt = consts.tile([128, bsh * T], BF16, tag="iota")
                nc.sync.dma_start(iota_bt[:], iota_d.ap())
                trans_sb = consts.tile([T, T], F32, tag="trans")
                nc.sync.dma_start(trans_sb[:], trans_d.ap())
                cnt_sb = consts.tile([T, T], F32, tag="cnt")
                nc.sync.dma_start(cnt_sb[:], cnt_d.ap())
                tags_sb = consts.tile([128, n_chunks * bsh], BF16, tag="tags")
                nc.sync.dma_start(tags_sb[:], tags_d.ap())
                tagsf_sb = consts.tile([128, n_chunks * bsh], F32,
                                       tag="tagsf")
                nc.sync.dma_start(tagsf_sb[:], tagsf_d.ap())
                blocksel = consts.tile([2, 128], BF16, tag="blocksel")
                nc.sync.dma_start(blocksel[:], bsel_d.ap())
                blockones = consts.tile([128, 2], BF16, tag="blockones")
                nc.sync.dma_start(blockones[:], bones_d.ap())

                # block-diagonal lhsT (bf16): top-left W (for W^T @ E),
                # bottom-right W^T (for W @ C)
                blockw = consts.tile([128, 128], BF16, tag="blockw")
                nc.vector.memset(blockw[:], 0.0)
                nc.scalar.activation(blockw[0:T, 0:T], trans_sb[:], AF.Exp)
                nc.scalar.activation(blockw[T:128, T:128], transt_sb[:],
                                     AF.Exp)

                ones64 = consts.tile([T, 1], F32, tag="ones64")
                nc.vector.memset(ones64[:], 1.0)
                negc0 = consts.tile([128, 1], F32, tag="negc0")
                nc.vector.memset(negc0[:], -C0)

                racc = consts.tile([2, n_ren * bsh], BF16, tag="racc")
                asum = consts.tile([128, n_chunks * num_slabs], F32,
                                   tag="asum")
                tsum = consts.tile([T, 1], F32, tag="tsum")
                nc.gpsimd.memset(racc[:], 1.0)
                nc.gpsimd.memset(asum[:], 0.0)
                nc.gpsimd.memset(tsum[:], 0.0)

                # numerator: trans part (one fused dot with host counts)
                def tsum_quantum():
                    scr3 = numscr.tile([T, T], F32, tag="nscr32")
                    nv.scalar_tensor_tensor(
                        scr3[:], cnt_sb[:], 1.0, trans_sb[:],
                        op0=ALU.bypass, op1=ALU.mult, accum_out=tsum[:])

                emg = {}      # em-chunk g -> tile [128, bsh, T] bf16
                ohmap = {}    # em-chunk g -> last one-hot slab tile
                exraw = {}    # x-chunk c -> tile [128, 128, bsh] bf16

                def dma_chunk(d):
                    xr = xrawpool.tile([128, 128, bsh], BF16, tag="xr")
                    nc.sync.dma_start(xr[:], em_x_d.ap()[d])
                    exraw[d] = xr
                    for g in (d, n_chunks - 1 - d):
                        eg = empool.tile([128, bsh, T], BF16, tag="em")
                        nc.sync.dma_start(eg[:], em_m_d.ap()[g])
                        emg[g] = eg

                def num_quanta(g):
                    """Numerator emission-part quanta for em chunk g:
                    num_slabs x (one-hot, fused mul-accum) over b-slabs."""
                    qs = []
                    if no_num:
                        return qs
                    for k in range(num_slabs):
                        def q_oh(g=g, k=k):
                            oh = numscr.tile([128, slab_b * T], BF16,
                                             tag="oh")
                            if num_pool_ts:
                                for bb in range(slab_b):
                                    col = g * bsh + k * slab_b + bb
                                    nc.gpsimd.tensor_scalar(
                                        oh[:, bb * T:(bb + 1) * T],
                                        iota_bt[:, 0:T],
                                        tagsf_sb[:, col:col + 1],
                                        None, op0=ALU.is_equal)
                            elif num_bcast:
                                tag_b = (tags_sb[:, g * bsh + k * slab_b:
                                                 g * bsh + (k + 1) * slab_b]
                                         .unsqueeze(2)
                                         .broadcast_to([128, slab_b, T]))
                                nv.tensor_tensor(
                                    oh[:], iota_bt[:, 0:slab_b * T],
                                    tag_b, op=ALU.is_equal)
                            else:
                                for bb in range(slab_b):
                                    nv.tensor_scalar(
                                        oh[:, bb * T:(bb + 1) * T],
                                        iota_bt[:, 0:T],
                                        tags_sb[:, g * bsh + k * slab_b + bb:
                                                g * bsh + k * slab_b + bb + 1],
                                        None, op0=ALU.is_equal)
                            ohmap[g] = oh
                        def q_acc(g=g, k=k, fin=(k == num_slabs - 1)):
                            scr = numscr.tile([128, slab_b * T], BF16,
                                              tag="nscr")
                            col = g * num_slabs + k
                            nv.scalar_tensor_tensor(
                                scr[:], ohmap[g][:], 1.0,
                                emg[g][:, k * slab_b:(k + 1) * slab_b, :],
                                op0=ALU.bypass, op1=ALU.mult,
                                accum_out=asum[:, col:col + 1])
                            if fin:
                                del emg[g]
                                del ohmap[g]
                        qs.append(q_oh)
                        qs.append(q_acc)
                    return qs

                def x_quanta(c):
                    """ACT-exp quanta producing X chunk c from em_x."""
                    xc = xpool.tile([128, 128, bsh], F32, tag="xc")
                    if fake_x:
                        def q():
                            nc.gpsimd.memset(xc[:], 0.0133)
                        return xc, [q]
                    qs = []
                    for hj in range(4):
                        def q(hj=hj):
                            sl = slice(hj * 32, (hj + 1) * 32)
                            nc.scalar.activation(
                                xc[:, sl, :], exraw[c][:, sl, :],
                                AF.Exp, bias=negc0[:])
                            if hj == 3:
                                del exraw[c]
                        qs.append(q)
                    return xc, qs

                # ---------------- main pipeline ----------------
                from collections import deque
                bg = deque()
                xchunks = {}
                if not fake_x:
                    dma_chunk(0)
                    if half > 1:
                        dma_chunk(1)
                    if half > 2:
                        dma_chunk(2)
                # prime only the first exp slab inline; the rest drain
                # through bg one quantum per round
                xc, qs = x_quanta(0)
                qs[0]()
                bg.extend(qs[1:])
                xchunks[0] = xc
                if half > 1:
                    xc, qs = x_quanta(1)
                    bg.extend(qs)
                    xchunks[1] = xc


                state = spool.tile([128, bsh], BF16, tag="st")
                nc.vector.tensor_copy(state[:], xchunks[0][:, 0, :])

                ren_set = set(ren_rounds)
                pending = {}   # round -> (xscr tile written, c, j)
                rbc_todo = {}  # round -> (p tile, ren index)
                ren_idx = 0

                for r in range(1, n_rounds + 1):
                    c, j = r >> 7, r & 127
                    if j == 1:
                        if not fake_x and c + 3 <= half - 1:
                            dma_chunk(c + 3)
                        if c + 2 <= half - 1:
                            xc, qs = x_quanta(c + 2)
                            xchunks[c + 2] = xc
                            bg.extend(qs)
                            xchunks.pop(c - 1, None)
                    elif j == 64:
                        if r == 64:
                            bg.append(tsum_quantum)
                        if not fake_x:
                            bg.extend(num_quanta(c))
                            bg.extend(num_quanta(n_chunks - 1 - c))
                    if bg:
                        bg.popleft()()

                    # delayed renorm: broadcast rhat (2 rounds after log)
                    if r in rbc_todo:
                        rm, k = rbc_todo.pop(r)
                        rbc = prbc.tile([128, bsh], F32, tag="rbc")
                        nc.tensor.matmul(
                            rbc[:], blocksel[:],
                            racc[:, k * bsh:(k + 1) * bsh],
                            start=True, stop=True)
                        pending[rm] = rbc

                    xsrc = xchunks[c][:, j, :]
                    if r in pending:
                        rbc = pending.pop(r)
                        xscr = smallpool.tile([128, bsh], F32, tag="xs")
                        ew.tensor_mul(xscr[:], xsrc, rbc[:])
                        xsrc = xscr[:]

                    p = pround.tile([128, bsh], F32, tag="p")
                    nc.tensor.matmul(p[:], blockw[:], state[:],
                                     start=True, stop=True)
                    if pe_warm == 1:
                        pd = pdummy.tile([128, 32], F32, tag="pd")
                        nc.tensor.matmul(pd[:, 0:1], blockw[:],
                                         blockw[:, 0:1],
                                         start=True, stop=True)
                        nc.tensor.matmul(pd[:, 0:1], blockw[:],
                                         blockw[:, 0:1],
                                         start=True, stop=True)
                    elif pe_warm == 2:
                        pd = pdummy.tile([128, 192], F32, tag="pd")
                        nc.tensor.matmul(pd[:], blockw[:],
                                         iota_bt[:, 0:192],
                                         start=True, stop=True)
                    state = spool.tile([128, bsh], BF16, tag="st")
                    ew.tensor_mul(state[:], p[:], xsrc)

                    if r in ren_set:
                        # per-direction mass onto partitions 0:2, then
                        # log its bf16 reciprocal into the racc ring
                        k = ren_idx
                        ren_idx += 1
                        mass = pmass.tile([2, bsh], F32, tag="mass")
                        nc.tensor.matmul(mass[:], blockones[:], state[:],
                                         start=True, stop=True)
                        nc.vector.reciprocal(
                            racc[:, k * bsh:(k + 1) * bsh], mass[:])
                        rbc_todo[r + 2] = (r + DELTA, k)

                while bg:
                    bg.popleft()()

                # ---------------- final combine ----------------
                # beta = W @ C on partitions 0..63 (aligned base-64 matmul)
                pf = pround.tile([128, bsh], F32, tag="p")
                nc.tensor.matmul(pf[0:T, :], blockw[T:128, T:128],
                                 state[T:128, :], start=True, stop=True)
                y = smallpool.tile([T, bsh], F32, tag="y")
                nc.vector.tensor_mul(y[:], state[0:T, :], pf[0:T, :])
                z = prbc.tile([128, bsh], F32, tag="rbc")
                nc.tensor.matmul(z[0:1, :], ones64[:], y[:],
                                 start=True, stop=True)
                z_sb = smallpool.tile([1, bsh], F32, tag="zsb")
                nc.vector.tensor_copy(z_sb[:], z[0:1, :])
                nc.sync.dma_start(z_d.ap(), z_sb[:])
                nc.sync.dma_start(racc_d.ap(), racc[:])
                nc.sync.dma_start(asum_d.ap(), asum[:])
                nc.sync.dma_start(tsum_d.ap(), tsum[:])

    nc.compile()
    return nc


def _get_nc(n_chunks=16, bsh=BSH):
    key = (n_chunks, bsh)
    if key not in _NC_CACHE:
        _NC_CACHE[key] = build(n_chunks, bsh)
    return _NC_CACHE[key]


def _consts(n_chunks=16, bsh=BSH):
    # iota_bt[s, b*T + t] = t
    iota = np.broadcast_to(np.arange(T, dtype=F32_NP)[None, None, :],
                           (128, bsh, T)).reshape(128, bsh * T)
    iota = np.ascontiguousarray(iota).astype(BF16_NP)
    bsel = np.zeros((2, 128), dtype=F32_NP)
    bsel[0, 0:T] = 1.0
    bsel[1, T:128] = 1.0
    bones = np.zeros((128, 2), dtype=F32_NP)
    bones[0:T, 0] = 1.0
    bones[T:128, 1] = 1.0
    return iota, bsel.astype(BF16_NP), bones.astype(BF16_NP)


def make_in_maps(emissions, start_transitions, end_transitions, transitions,
                 tags, ncores=NCORES):
    """Host prep: fold start/end into em, convert to bf16, build the two
    DMA-friendly layouts (em_x for the recurrence, em_m for the
    numerator), pair-count matrix from tags, shard over cores."""
    em = np.asarray(emissions, dtype=F32_NP).copy()
    em[:, 0, :] += np.asarray(start_transitions, dtype=F32_NP)
    em[:, -1, :] += np.asarray(end_transitions, dtype=F32_NP)
    em_b = em.astype(BF16_NP)
    b_all, s_len = em.shape[0], em.shape[1]
    n_chunks = s_len // CHUNK
    half = n_chunks // 2
    # em_x[c, row, j, b]: rows 0:64 fwd t of chunk c (s = 128c + j);
    # rows 64:128 bwd t of chunk n_chunks-1-c with j reversed
    # (s = s_len-1 - 128c - j)
    fwd = em_b[:, :half * 128, :].reshape(b_all, half, 128, T)
    fwd = fwd.transpose(1, 3, 2, 0)                    # [c, t, j, b]
    bwd = em_b[:, half * 128:, :].reshape(b_all, half, 128, T)
    bwd = bwd[:, ::-1, ::-1, :].transpose(1, 3, 2, 0)  # [c, t, j, b]
    em_x = np.concatenate([fwd, bwd], axis=1)          # [c, 128, 128, b]
    # em_m[g, s, b, t]
    em_m = em_b.reshape(b_all, n_chunks, 128, T).transpose(1, 2, 0, 3)
    tags_i = np.asarray(tags).astype(np.int64).reshape(b_all, s_len)
    # tags_t[s, g, b] = tags[b, 128g + s]  (bf16; 0..63 exact)
    tags_t = (tags_i.reshape(b_all, n_chunks, CHUNK)
              .transpose(2, 1, 0).astype(F32_NP))
    trans = np.asarray(transitions, dtype=F32_NP).reshape(T, T)
    iota, bsel, bones = _consts(n_chunks)
    bsh = b_all // ncores
    in_maps = []
    for cidx in range(ncores):
        sl = slice(cidx * bsh, (cidx + 1) * bsh)
        # pair counts from this core's tags (index data only)
        pair = (T * tags_i[sl, :-1] + tags_i[sl, 1:]).ravel()
        cnt = np.bincount(pair, minlength=T * T).astype(F32_NP)
        in_maps.append({
            "emx": np.ascontiguousarray(em_x[:, :, :, sl]),
            "emm": np.ascontiguousarray(em_m[:, :, sl, :]),
            "tagst": np.ascontiguousarray(
                tags_t[:, :, sl].reshape(CHUNK, n_chunks * bsh)
            ).astype(BF16_NP),
            "tagsf": np.ascontiguousarray(
                tags_t[:, :, sl].reshape(CHUNK, n_chunks * bsh)),
            "cnt": cnt.reshape(T, T),
            "trans": trans,
            "transt": np.ascontiguousarray(trans.T),
            "bsel": bsel,
            "bones": bones,
            "iotat": iota,
        })
    return in_maps


def kernel(emissions, start_transitions, end_transitions, transitions,
           tags, mask):
    """Full-input entry point; shards over 8 NeuronCores internally."""
    from concourse.bass_utils import run_bass_kernel_spmd

    emissions = np.asarray(emissions)
    assert emissions.shape == (B, S, T)
    assert (np.asarray(mask) != 0).all(), "kernel assumes all-ones mask"

    in_maps = make_in_maps(emissions, start_transitions, end_transitions,
                           transitions, tags)
    nc = _get_nc()
    res = run_bass_kernel_spmd(nc, in_maps, core_ids=list(range(NCORES)))

    num_total = 0.0
    den_total = 0.0
    for cidx in range(NCORES):
        r = res.results[cidx]
        num_total += float(r["asum"].sum()) + float(r["tsum"].sum())
        den = (np.log(r["z"].astype(np.float64))[0]
               + S * C0
               - np.log(r["racc"].astype(np.float64)
                        .reshape(2, -1, BSH)).sum(axis=(0, 1)))
        den_total += float(den.sum())
    loss = -(num_total - den_total) / float(B)
    return np.float32(loss)


# revision 22
# speedup vs baseline: 1.4406x; 1.4406x over previous
"""CRF negative-log-likelihood loss kernel for Trainium2 (8 NeuronCores).

Problem: B=256, S=2048, T=64 CRF loss (torchcrf-style), mask all-ones.

Strategy (v2)
-------------
Data-parallel over batch: each of the 8 cores gets 32 batch rows.

Denominator (log-partition): forward/backward meet-in-the-middle.  The
forward chain  E_p = X_p * (W^T E_{p-1})  and the backward chain
C_s = X_s * (W C_{s+1})  (exp domain, X_s = exp(em_s - c0), W =
exp(trans)) advance together: one 128x128 block-diagonal matmul (top
block W, bottom block W^T as lhsT, bf16) + one [128,32] DVE multiply
per round.  1023 rounds instead of 2047.  Z = E_{S/2-1} . (W C_{S/2}).

Renormalization (v2, off the critical path): every RN rounds a tiny
blockones matmul sums each direction's 64 state rows onto partitions
0:2; the DVE logs the bf16 reciprocals into the racc ring (exact
applied values -> host adds sum(log) back).  Two rounds later a
blocksel PE matmul broadcasts them to 128 rows, and DELTA rounds after
the log they are applied by pre-scaling that round's x-slice on the
DVE (scale-by-column commutes with the matmul).  Nothing on the
chain's PE->DVE->PE path ever waits on ACT/Pool, and no per-renorm
Ln/log work happens on device.

Numerator (v2):
  trans part: host computes the pair-count matrix cnt[t,t'] from tags
    (index data only); device does one fused multiply-accumulate with
    trans.  -> 1 DVE instruction.
  emission part: per seq-chunk, one-hot oh[s,t,b] = (iota_tb == tag
    broadcast) on DVE (bf16, 2x mode), then fused (oh * em) with
    per-partition accumulation into asum columns.  Split into 4 b-slabs
    per chunk (one-hots further halved) so no DVE instruction greatly
    exceeds the chain's idle window (no chain stalls).
  start/end transitions are folded into em rows s=0 / s=S-1 on the
  host (also makes X_0 / X_{S-1} the correct chain initializers).

Device outputs are raw (z, renorm ring, asum/tsum partials); the final
logs/sums are host-side scalar work on tiny tensors.

Per-core outputs: z[1,32] f32, racc[2,NRN,32] bf16, asum[128,64] f32,
tsum[64,1] f32.  Host: den_b = ln(z_b)+S*c0-sum_k ln(racc[:,k,b]);
loss = -(sum(asum)+sum(tsum) - sum(den)) / B.
"""

import contextlib

import numpy as np
import ml_dtypes

F32_NP = np.float32
BF16_NP = ml_dtypes.bfloat16

B, S, T = 256, 2048, 64
NCORES = 8
BSH = B // NCORES  # 32
CHUNK = 128
C0 = 4.8204  # ~ ln(64 * e^0.5 * sinh(1)) : expected per-step log growth
RN = 128     # renorm every RN rounds (drift stays << f32 range)
DELTA = 8    # rounds between logging a renorm scale and applying it

_NC_CACHE = {}


def build(n_chunks=16, bsh=BSH, nrep=1, rn=RN, no_num=False, fake_x=False,
          pround_bufs=4, spool_bufs=6, num_slabs=4, chain_on_pool=False,
          num_bcast=True, num_on_pool=False, num_pool_ts=False,
          pe_warm=0, oh_split=2):
    """Build + compile the per-core Bass module. n_chunks*128 = seq len."""
    import concourse.bacc as bacc
    import concourse.mybir as mybir
    import concourse.tile as tile

    F32 = mybir.dt.float32
    BF16 = mybir.dt.bfloat16
    AF = mybir.ActivationFunctionType
    ALU = mybir.AluOpType

    s_len = n_chunks * CHUNK
    half = n_chunks // 2
    assert half * 2 == n_chunks and half >= 1
    n_rounds = half * CHUNK - 1
    # renorm rounds: r = rn, 2*rn, ... with r + DELTA <= n_rounds
    ren_rounds = [r for r in range(rn, n_rounds + 1, rn) if r + DELTA <= n_rounds]
    n_ren = max(1, len(ren_rounds))
    slab_b = bsh // num_slabs  # batch-extent of one numerator slab

    nc = bacc.Bacc("TRN2", target_bir_lowering=False, debug=False,
                   num_devices=NCORES)

    em_x_d = nc.dram_tensor("emx", [half, 128, 128, bsh], BF16,
                            kind="ExternalInput")
    em_m_d = nc.dram_tensor("emm", [n_chunks, 128, bsh, T], BF16,
                            kind="ExternalInput")
    tags_d = nc.dram_tensor("tagst", [128, n_chunks * bsh], BF16,
                            kind="ExternalInput")
    tagsf_d = nc.dram_tensor("tagsf", [128, n_chunks * bsh], F32,
                            kind="ExternalInput")
    cnt_d = nc.dram_tensor("cnt", [T, T], F32, kind="ExternalInput")
    trans_d = nc.dram_tensor("trans", [T, T], F32, kind="ExternalInput")
    bsel_d = nc.dram_tensor("bsel", [2, 128], BF16, kind="ExternalInput")
    bones_d = nc.dram_tensor("bones", [128, 2], BF16, kind="ExternalInput")
    iota_d = nc.dram_tensor("iotat", [128, bsh * T], BF16,
                            kind="ExternalInput")
    transt_d = nc.dram_tensor("transt", [T, T], F32, kind="ExternalInput")
    z_d = nc.dram_tensor("z", [1, bsh], F32, kind="ExternalOutput")
    racc_d = nc.dram_tensor("racc", [2, n_ren * bsh], BF16,
                            kind="ExternalOutput")
    asum_d = nc.dram_tensor("asum", [128, n_chunks * num_slabs], F32,
                            kind="ExternalOutput")
    tsum_d = nc.dram_tensor("tsum", [T, 1], F32, kind="ExternalOutput")

    ew = nc.gpsimd if chain_on_pool else nc.vector
    nv = nc.gpsimd if num_on_pool else nc.vector

    with tile.TileContext(nc) as tc, nc.allow_low_precision(
            reason="bf16 state/weights validated against f64 reference"):
        with (
            tc.tile_pool(name="consts", bufs=1) as consts,
            tc.tile_pool(name="xchunk", bufs=3) as xpool,
            tc.tile_pool(name="xraw", bufs=3) as xrawpool,
            tc.tile_pool(name="emt", bufs=6) as empool,
            tc.tile_pool(name="numscr", bufs=4) as numscr,
            tc.tile_pool(name="state", bufs=spool_bufs) as spool,
            tc.tile_pool(name="small", bufs=4) as smallpool,
            tc.tile_pool(name="pround", bufs=pround_bufs,
                         space="PSUM") as pround,
            tc.tile_pool(name="prbc", bufs=2, space="PSUM") as prbc,
            tc.tile_pool(name="pmass", bufs=1, space="PSUM") as pmass,
            tc.tile_pool(name="pdummy", bufs=1, space="PSUM") as pdummy,
        ):
            rep_ctx = (tc.For_i(0, nrep, 1) if nrep > 1
                       else contextlib.nullcontext())
            with rep_ctx:
                # ---------------- constants / setup ----------------
                transt_sb = consts.tile([T, T], F32, tag="transt")
                nc.sync.dma_start(transt_sb[:], transt_d.ap())
                iota_bt = consts.tile([128, bsh * T], BF16, tag="iota")
                nc.sync.dma_start(iota_bt[:], iota_d.ap())
                trans_sb = consts.tile([T, T], F32, tag="trans")
                nc.sync.dma_start(trans_sb[:], trans_d.ap())
                cnt_sb = consts.tile([T, T], F32, tag="cnt")
                nc.sync.dma_start(cnt_sb[:], cnt_d.ap())
                tags_sb = consts.tile([128, n_chunks * bsh], BF16, tag="tags")
                nc.sync.dma_start(tags_sb[:], tags_d.ap())
                if num_pool_ts:
                    tagsf_sb = consts.tile([128, n_chunks * bsh], F32,
                                           tag="tagsf")
                    nc.sync.dma_start(tagsf_sb[:], tagsf_d.ap())
                blocksel = consts.tile([2, 128], BF16, tag="blocksel")
                nc.sync.dma_start(blocksel[:], bsel_d.ap())
                blockones = consts.tile([128, 2], BF16, tag="blockones")
                nc.sync.dma_start(blockones[:], bones_d.ap())

                # block-diagonal lhsT (bf16): top-left W (for W^T @ E),
                # bottom-right W^T (for W @ C)
                blockw = consts.tile([128, 128], BF16, tag="blockw")
                nc.vector.memset(blockw[:], 0.0)
                nc.scalar.activation(blockw[0:T, 0:T], trans_sb[:], AF.Exp)
                nc.scalar.activation(blockw[T:128, T:128], transt_sb[:],
                                     AF.Exp)

                ones64 = consts.tile([T, 1], F32, tag="ones64")
                nc.vector.memset(ones64[:], 1.0)
                negc0 = consts.tile([128, 1], F32, tag="negc0")
                nc.vector.memset(negc0[:], -C0)

                racc = consts.tile([2, n_ren * bsh], BF16, tag="racc")
                asum = consts.tile([128, n_chunks * num_slabs], F32,
                                   tag="asum")
                tsum = consts.tile([T, 1], F32, tag="tsum")
                nc.gpsimd.memset(racc[:], 1.0)
                nc.gpsimd.memset(asum[:], 0.0)
                nc.gpsimd.memset(tsum[:], 0.0)

                # numerator: trans part (one fused dot with host counts)
                def tsum_quantum():
                    scr3 = numscr.tile([T, T], F32, tag="nscr32")
                    nv.scalar_tensor_tensor(
                        scr3[:], cnt_sb[:], 1.0, trans_sb[:],
                        op0=ALU.bypass, op1=ALU.mult, accum_out=tsum[:])

                emg = {}      # em-chunk g -> tile [128, bsh, T] bf16
                ohmap = {}    # em-chunk g -> last one-hot slab tile
                exraw = {}    # x-chunk c -> tile [128, 128, bsh] bf16

                def dma_chunk(d):
                    xr = xrawpool.tile([128, 128, bsh], BF16, tag="xr")
                    nc.sync.dma_start(xr[:], em_x_d.ap()[d])
                    exraw[d] = xr
                    for g in (d, n_chunks - 1 - d):
                        eg = empool.tile([128, bsh, T], BF16, tag="em")
                        nc.sync.dma_start(eg[:], em_m_d.ap()[g])
                        emg[g] = eg

                def num_quanta(g):
                    """Numerator emission-part quanta for em chunk g:
                    num_slabs x (one-hot, fused mul-accum) over b-slabs."""
                    qs = []
                    if no_num:
                        return qs
                    for k in range(num_slabs):
                        def q_oh(g=g, k=k, part=None):
                            if part is None or part == 0:
                                oh = numscr.tile([128, slab_b * T], BF16,
                                                 tag="oh")
                                ohmap[g] = oh
                            else:
                                oh = ohmap[g]
                            if num_pool_ts:
                                for bb in range(slab_b):
                                    col = g * bsh + k * slab_b + bb
                                    nc.gpsimd.tensor_scalar(
                                        oh[:, bb * T:(bb + 1) * T],
                                        iota_bt[:, 0:T],
                                        tagsf_sb[:, col:col + 1],
                                        None, op0=ALU.is_equal)
                            elif num_bcast:
                                if part is None:
                                    lo, hi = 0, slab_b
                                else:
                                    w = slab_b // oh_split
                                    lo, hi = part * w, (part + 1) * w
                                tag_b = (tags_sb[:, g * bsh + k * slab_b + lo:
                                                 g * bsh + k * slab_b + hi]
                                         .unsqueeze(2)
                                         .broadcast_to([128, hi - lo, T]))
                                nv.tensor_tensor(
                                    oh[:, lo * T:hi * T],
                                    iota_bt[:, 0:(hi - lo) * T],
                                    tag_b, op=ALU.is_equal)
                            else:
                                for bb in range(slab_b):
                                    nv.tensor_scalar(
                                        oh[:, bb * T:(bb + 1) * T],
                                        iota_bt[:, 0:T],
                                        tags_sb[:, g * bsh + k * slab_b + bb:
                                                g * bsh + k * slab_b + bb + 1],
                                        None, op0=ALU.is_equal)
                        def q_acc(g=g, k=k, fin=(k == num_slabs - 1)):
                            scr = numscr.tile([128, slab_b * T], BF16,
                                              tag="nscr")
                            col = g * num_slabs + k
                            nv.scalar_tensor_tensor(
                                scr[:], ohmap[g][:], 1.0,
                                emg[g][:, k * slab_b:(k + 1) * slab_b, :],
                                op0=ALU.bypass, op1=ALU.mult,
                                accum_out=asum[:, col:col + 1])
                            if fin:
                                del emg[g]
                                del ohmap[g]
                        if num_bcast and not num_pool_ts and oh_split > 1:
                            for part in range(oh_split):
                                qs.append(
                                    lambda g=g, k=k, part=part:
                                    q_oh(g=g, k=k, part=part))
                        else:
                            qs.append(q_oh)
                        qs.append(q_acc)
                    return qs

                def x_quanta(c):
                    """ACT-exp quanta producing X chunk c from em_x."""
                    xc = xpool.tile([128, 128, bsh], F32, tag="xc")
                    if fake_x:
                        def q():
                            nc.gpsimd.memset(xc[:], 0.0133)
                        return xc, [q]
                    qs = []
                    for hj in range(4):
                        def q(hj=hj):
                            sl = slice(hj * 32, (hj + 1) * 32)
                            nc.scalar.activation(
                                xc[:, sl, :], exraw[c][:, sl, :],
                                AF.Exp, bias=negc0[:])
                            if hj == 3:
                                del exraw[c]
                        qs.append(q)
                    return xc, qs

                # ---------------- main pipeline ----------------
                from collections import deque
                bg = deque()
                xchunks = {}
                if not fake_x:
                    dma_chunk(0)
                    if half > 1:
                        dma_chunk(1)
                    if half > 2:
                        dma_chunk(2)
                # prime only the first exp slab inline; the rest drain
                # through bg one quantum per round
                xc, qs = x_quanta(0)
                qs[0]()
                bg.extend(qs[1:])
                xchunks[0] = xc
                if half > 1:
                    xc, qs = x_quanta(1)
                    bg.extend(qs)
                    xchunks[1] = xc

                state = spool.tile([128, bsh], BF16, tag="st")
                nc.vector.tensor_copy(state[:], xchunks[0][:, 0, :])

                ren_set = set(ren_rounds)
                pending = {}   # round -> (xscr tile written, c, j)
                rbc_todo = {}  # round -> (p tile, ren index)
                ren_idx = 0

                for r in range(1, n_rounds + 1):
                    c, j = r >> 7, r & 127
                    if j == 1:
                        if not fake_x and c + 3 <= half - 1:
                            dma_chunk(c + 3)
                        if c + 2 <= half - 1:
                            xc, qs = x_quanta(c + 2)
                            xchunks[c + 2] = xc
                            bg.extend(qs)
                            xchunks.pop(c - 1, None)
                    elif j == 64:
                        if r == 64:
                            bg.append(tsum_quantum)
                        if not fake_x:
                            bg.extend(num_quanta(c))
                            bg.extend(num_quanta(n_chunks - 1 - c))
                    if bg:
                        bg.popleft()()

                    # delayed renorm: broadcast rhat (2 rounds after log)
                    if r in rbc_todo:
                        rm, k = rbc_todo.pop(r)
                        rbc = prbc.tile([128, bsh], F32, tag="rbc")
                        nc.tensor.matmul(
                            rbc[:], blocksel[:],
                            racc[:, k * bsh:(k + 1) * bsh],
                            start=True, stop=True)
                        pending[rm] = rbc

                    xsrc = xchunks[c][:, j, :]
                    if r in pending:
                        rbc = pending.pop(r)
                        xscr = smallpool.tile([128, bsh], F32, tag="xs")
                        ew.tensor_mul(xscr[:], xsrc, rbc[:])
                        xsrc = xscr[:]

                    p = pround.tile([128, bsh], F32, tag="p")
                    nc.tensor.matmul(p[:], blockw[:], state[:],
                                     start=True, stop=True)
                    if pe_warm == 1:
                        pd = pdummy.tile([128, 32], F32, tag="pd")
                        nc.tensor.matmul(pd[:, 0:1], blockw[:],
                                         blockw[:, 0:1],
                                         start=True, stop=True)
                        nc.tensor.matmul(pd[:, 0:1], blockw[:],
                                         blockw[:, 0:1],
                                         start=True, stop=True)
                    elif pe_warm == 2:
                        pd = pdummy.tile([128, 192], F32, tag="pd")
                        nc.tensor.matmul(pd[:], blockw[:],
                                         iota_bt[:, 0:192],
                                         start=True, stop=True)
                    state = spool.tile([128, bsh], BF16, tag="st")
                    ew.tensor_mul(state[:], p[:], xsrc)

                    if r in ren_set:
                        # per-direction mass onto partitions 0:2, then
                        # log its bf16 reciprocal into the racc ring
                        k = ren_idx
                        ren_idx += 1
                        mass = pmass.tile([2, bsh], F32, tag="mass")
                        nc.tensor.matmul(mass[:], blockones[:], state[:],
                                         start=True, stop=True)
                        nc.vector.reciprocal(
                            racc[:, k * bsh:(k + 1) * bsh], mass[:])
                        rbc_todo[r + 2] = (r + DELTA, k)

                while bg:
                    bg.popleft()()

                # ---------------- final combine ----------------
                # beta = W @ C on partitions 0..63 (aligned base-64 matmul)
                pf = pround.tile([128, bsh], F32, tag="p")
                nc.tensor.matmul(pf[0:T, :], blockw[T:128, T:128],
                                 state[T:128, :], start=True, stop=True)
                y = smallpool.tile([T, bsh], F32, tag="y")
                nc.vector.tensor_mul(y[:], state[0:T, :], pf[0:T, :])
                z = prbc.tile([128, bsh], F32, tag="rbc")
                nc.tensor.matmul(z[0:1, :], ones64[:], y[:],
                                 start=True, stop=True)
                z_sb = smallpool.tile([1, bsh], F32, tag="zsb")
                nc.vector.tensor_copy(z_sb[:], z[0:1, :])
                nc.sync.dma_start(z_d.ap(), z_sb[:])
                nc.sync.dma_start(racc_d.ap(), racc[:])
                nc.sync.dma_start(asum_d.ap(), asum[:])
                nc.sync.dma_start(tsum_d.ap(), tsum[:])

    nc.compile()
    return nc


def _get_nc(n_chunks=16, bsh=BSH):
    key = (n_chunks, bsh)
    if key not in _NC_CACHE:
        _NC_CACHE[key] = build(n_chunks, bsh)
    return _NC_CACHE[key]


def _consts(n_chunks=16, bsh=BSH):
    # iota_bt[s, b*T + t] = t
    iota = np.broadcast_to(np.arange(T, dtype=F32_NP)[None, None, :],
                           (128, bsh, T)).reshape(128, bsh * T)
    iota = np.ascontiguousarray(iota).astype(BF16_NP)
    bsel = np.zeros((2, 128), dtype=F32_NP)
    bsel[0, 0:T] = 1.0
    bsel[1, T:128] = 1.0
    bones = np.zeros((128, 2), dtype=F32_NP)
    bones[0:T, 0] = 1.0
    bones[T:128, 1] = 1.0
    return iota, bsel.astype(BF16_NP), bones.astype(BF16_NP)


def make_in_maps(emissions, start_transitions, end_transitions, transitions,
                 tags, ncores=NCORES):
    """Host prep: fold start/end into em, convert to bf16, build the two
    DMA-friendly layouts (em_x for the recurrence, em_m for the
    numerator), pair-count matrix from tags, shard over cores."""
    em = np.asarray(emissions, dtype=F32_NP).copy()
    em[:, 0, :] += np.asarray(start_transitions, dtype=F32_NP)
    em[:, -1, :] += np.asarray(end_transitions, dtype=F32_NP)
    em_b = em.astype(BF16_NP)
    b_all, s_len = em.shape[0], em.shape[1]
    n_chunks = s_len // CHUNK
    half = n_chunks // 2
    # em_x[c, row, j, b]: rows 0:64 fwd t of chunk c (s = 128c + j);
    # rows 64:128 bwd t of chunk n_chunks-1-c with j reversed
    # (s = s_len-1 - 128c - j)
    fwd = em_b[:, :half * 128, :].reshape(b_all, half, 128, T)
    fwd = fwd.transpose(1, 3, 2, 0)                    # [c, t, j, b]
    bwd = em_b[:, half * 128:, :].reshape(b_all, half, 128, T)
    bwd = bwd[:, ::-1, ::-1, :].transpose(1, 3, 2, 0)  # [c, t, j, b]
    em_x = np.concatenate([fwd, bwd], axis=1)          # [c, 128, 128, b]
    # em_m[g, s, b, t]
    em_m = em_b.reshape(b_all, n_chunks, 128, T).transpose(1, 2, 0, 3)
    tags_i = np.asarray(tags).astype(np.int64).reshape(b_all, s_len)
    # tags_t[s, g, b] = tags[b, 128g + s]  (bf16; 0..63 exact)
    tags_t = (tags_i.reshape(b_all, n_chunks, CHUNK)
              .transpose(2, 1, 0).astype(F32_NP))
    trans = np.asarray(transitions, dtype=F32_NP).reshape(T, T)
    iota, bsel, bones = _consts(n_chunks)
    bsh = b_all // ncores
    in_maps = []
    for cidx in range(ncores):
        sl = slice(cidx * bsh, (cidx + 1) * bsh)
        # pair counts from this core's tags (index data only)
        pair = (T * tags_i[sl, :-1] + tags_i[sl, 1:]).ravel()
        cnt = np.bincount(pair, minlength=T * T).astype(F32_NP)
        in_maps.append({
            "emx": np.ascontiguousarray(em_x[:, :, :, sl]),
            "emm": np.ascontiguousarray(em_m[:, :, sl, :]),
            "tagst": np.ascontiguousarray(
                tags_t[:, :, sl].reshape(CHUNK, n_chunks * bsh)
            ).astype(BF16_NP),
            "tagsf": np.ascontiguousarray(
                tags_t[:, :, sl].reshape(CHUNK, n_chunks * bsh)),
            "cnt": cnt.reshape(T, T),
            "trans": trans,
            "transt": np.ascontiguousarray(trans.T),
            "bsel": bsel,
            "bones": bones,
            "iotat": iota,
        })
    return in_maps


def kernel(emissions, start_transitions, end_transitions, transitions,
           tags, mask):
    """Full-input entry point; shards over 8 NeuronCores internally."""
    from concourse.bass_utils import run_bass_kernel_spmd

    emissions = np.asarray(emissions)
    assert emissions.shape == (B, S, T)
    assert (np.asarray(mask) != 0).all(), "kernel assumes all-ones mask"

    in_maps = make_in_maps(emissions, start_transitions, end_transitions,
                           transitions, tags)
    nc = _get_nc()
    res = run_bass_kernel_spmd(nc, in_maps, core_ids=list(range(NCORES)))

    num_total = 0.0
    den_total = 0.0
    for cidx in range(NCORES):
        r = res.results[cidx]
        num_total += float(r["asum"].sum()) + float(r["tsum"].sum())
        den = (np.log(r["z"].astype(np.float64))[0]
               + S * C0
               - np.log(r["racc"].astype(np.float64)
                        .reshape(2, -1, BSH)).sum(axis=(0, 1)))
        den_total += float(den.sum())
    loss = -(num_total - den_total) / float(B)
    return np.float32(loss)



# revision 23
# speedup vs baseline: 1.5665x; 1.0874x over previous
"""CRF negative-log-likelihood loss kernel for Trainium2 (8 NeuronCores).

Problem: B=256, S=2048, T=64 CRF loss (torchcrf-style), mask all-ones.

Strategy (v2)
-------------
Data-parallel over batch: each of the 8 cores gets 32 batch rows.

Denominator (log-partition): forward/backward meet-in-the-middle.  The
forward chain  E_p = X_p * (W^T E_{p-1})  and the backward chain
C_s = X_s * (W C_{s+1})  (exp domain, X_s = exp(em_s - c0), W =
exp(trans)) advance together: one 128x128 block-diagonal matmul (top
block W, bottom block W^T as lhsT, bf16) + one [128,32] DVE multiply
per round.  1023 rounds instead of 2047.  Z = E_{S/2-1} . (W C_{S/2}).

Renormalization (v2, off the critical path): every RN rounds a tiny
blockones matmul sums each direction's 64 state rows onto partitions
0:2; the DVE logs the bf16 reciprocals into the racc ring (exact
applied values -> host adds sum(log) back).  Two rounds later a
blocksel PE matmul broadcasts them to 128 rows, and DELTA rounds after
the log they are applied by pre-scaling that round's x-slice on the
DVE (scale-by-column commutes with the matmul).  Nothing on the
chain's PE->DVE->PE path ever waits on ACT/Pool, and no per-renorm
Ln/log work happens on device.

Numerator (v2):
  trans part: host computes the pair-count matrix cnt[t,t'] from tags
    (index data only); device does one fused multiply-accumulate with
    trans.  -> 1 DVE instruction.
  emission part: per seq-chunk, one-hot oh[s,t,b] = (iota_tb == tag
    broadcast) on DVE (bf16, 2x mode), then fused (oh * em) with
    per-partition accumulation into asum columns.  Split into 4 b-slabs
    per chunk (one-hots further halved) so no DVE instruction greatly
    exceeds the chain's idle window (no chain stalls).
  start/end transitions are folded into em rows s=0 / s=S-1 on the
  host (also makes X_0 / X_{S-1} the correct chain initializers).

Device outputs are raw (z, renorm ring, asum/tsum partials); the final
logs/sums are host-side scalar work on tiny tensors.

Per-core outputs: z[1,32] f32, racc[2,NRN,32] bf16, asum[128,64] f32,
tsum[64,1] f32.  Host: den_b = ln(z_b)+S*c0-sum_k ln(racc[:,k,b]);
loss = -(sum(asum)+sum(tsum) - sum(den)) / B.
"""

import contextlib

import numpy as np
import ml_dtypes

F32_NP = np.float32
BF16_NP = ml_dtypes.bfloat16

B, S, T = 256, 2048, 64
NCORES = 8
BSH = B // NCORES  # 32
CHUNK = 128
C0 = 4.8204  # ~ ln(64 * e^0.5 * sinh(1)) : expected per-step log growth
RN = 128     # renorm every RN rounds (drift stays << f32 range)
DELTA = 8    # rounds between logging a renorm scale and applying it

_NC_CACHE = {}


def build(n_chunks=16, bsh=BSH, nrep=1, rn=RN, no_num=False, fake_x=False,
          pround_bufs=4, spool_bufs=6, num_slabs=4, chain_on_pool=False,
          num_bcast=True, num_on_pool=False, num_pool_ts=False,
          pe_warm=0, oh_split=2, ren_inplace=False):
    """Build + compile the per-core Bass module. n_chunks*128 = seq len."""
    import concourse.bacc as bacc
    import concourse.mybir as mybir
    import concourse.tile as tile

    F32 = mybir.dt.float32
    BF16 = mybir.dt.bfloat16
    AF = mybir.ActivationFunctionType
    ALU = mybir.AluOpType

    s_len = n_chunks * CHUNK
    half = n_chunks // 2
    assert half * 2 == n_chunks and half >= 1
    n_rounds = half * CHUNK - 1
    # renorm rounds: r = rn, 2*rn, ... with r + DELTA <= n_rounds
    ren_rounds = [r for r in range(rn, n_rounds + 1, rn) if r + DELTA <= n_rounds]
    n_ren = max(1, len(ren_rounds))
    slab_b = bsh // num_slabs  # batch-extent of one numerator slab

    nc = bacc.Bacc("TRN2", target_bir_lowering=False, debug=False,
                   num_devices=NCORES)

    em_x_d = nc.dram_tensor("emx", [half, 128, 128, bsh], BF16,
                            kind="ExternalInput")
    em_m_d = nc.dram_tensor("emm", [n_chunks, 128, bsh, T], BF16,
                            kind="ExternalInput")
    tags_d = nc.dram_tensor("tagst", [128, n_chunks * bsh], BF16,
                            kind="ExternalInput")
    tagsf_d = nc.dram_tensor("tagsf", [128, n_chunks * bsh], F32,
                            kind="ExternalInput")
    cnt_d = nc.dram_tensor("cnt", [T, T], F32, kind="ExternalInput")
    trans_d = nc.dram_tensor("trans", [T, T], F32, kind="ExternalInput")
    bsel_d = nc.dram_tensor("bsel", [2, 128], BF16, kind="ExternalInput")
    bones_d = nc.dram_tensor("bones", [128, 2], BF16, kind="ExternalInput")
    sel128_d = nc.dram_tensor("sel128", [128, 128], BF16,
                              kind="ExternalInput")
    iota_d = nc.dram_tensor("iotat", [128, bsh * T], BF16,
                            kind="ExternalInput")
    transt_d = nc.dram_tensor("transt", [T, T], F32, kind="ExternalInput")
    z_d = nc.dram_tensor("z", [1, bsh], F32, kind="ExternalOutput")
    racc_d = nc.dram_tensor("racc", [2, n_ren * bsh], BF16,
                            kind="ExternalOutput")
    asum_d = nc.dram_tensor("asum", [128, n_chunks * num_slabs], F32,
                            kind="ExternalOutput")
    tsum_d = nc.dram_tensor("tsum", [T, 1], F32, kind="ExternalOutput")

    ew = nc.gpsimd if chain_on_pool else nc.vector
    nv = nc.gpsimd if num_on_pool else nc.vector

    with tile.TileContext(nc) as tc, nc.allow_low_precision(
            reason="bf16 state/weights validated against f64 reference"):
        with (
            tc.tile_pool(name="consts", bufs=1) as consts,
            tc.tile_pool(name="xchunk", bufs=3) as xpool,
            tc.tile_pool(name="xraw", bufs=3) as xrawpool,
            tc.tile_pool(name="emt", bufs=6) as empool,
            tc.tile_pool(name="numscr", bufs=4) as numscr,
            tc.tile_pool(name="state", bufs=spool_bufs) as spool,
            tc.tile_pool(name="small", bufs=4) as smallpool,
            tc.tile_pool(name="pround", bufs=pround_bufs,
                         space="PSUM") as pround,
            tc.tile_pool(name="prbc", bufs=2, space="PSUM") as prbc,
            tc.tile_pool(name="pmass", bufs=1, space="PSUM") as pmass,
            tc.tile_pool(name="pdummy", bufs=1, space="PSUM") as pdummy,
        ):
            rep_ctx = (tc.For_i(0, nrep, 1) if nrep > 1
                       else contextlib.nullcontext())
            with rep_ctx:
                # ---------------- constants / setup ----------------
                transt_sb = consts.tile([T, T], F32, tag="transt")
                nc.sync.dma_start(transt_sb[:], transt_d.ap())
                iota_bt = consts.tile([128, bsh * T], BF16, tag="iota")
                nc.sync.dma_start(iota_bt[:], iota_d.ap())
                trans_sb = consts.tile([T, T], F32, tag="trans")
                nc.sync.dma_start(trans_sb[:], trans_d.ap())
                cnt_sb = consts.tile([T, T], F32, tag="cnt")
                nc.sync.dma_start(cnt_sb[:], cnt_d.ap())
                tags_sb = consts.tile([128, n_chunks * bsh], BF16, tag="tags")
                nc.sync.dma_start(tags_sb[:], tags_d.ap())
                if num_pool_ts:
                    tagsf_sb = consts.tile([128, n_chunks * bsh], F32,
                                           tag="tagsf")
                    nc.sync.dma_start(tagsf_sb[:], tagsf_d.ap())
                blocksel = consts.tile([2, 128], BF16, tag="blocksel")
                nc.sync.dma_start(blocksel[:], bsel_d.ap())
                blockones = consts.tile([128, 2], BF16, tag="blockones")
                nc.sync.dma_start(blockones[:], bones_d.ap())
                if ren_inplace:
                    sel128 = consts.tile([128, 128], BF16, tag="sel128")
                    nc.sync.dma_start(sel128[:], sel128_d.ap())
                    racc128 = consts.tile([128, n_ren * bsh], BF16,
                                          tag="racc128")
                    nc.gpsimd.memset(racc128[:], 1.0)

                # block-diagonal lhsT (bf16): top-left W (for W^T @ E),
                # bottom-right W^T (for W @ C)
                blockw = consts.tile([128, 128], BF16, tag="blockw")
                nc.vector.memset(blockw[:], 0.0)
                nc.scalar.activation(blockw[0:T, 0:T], trans_sb[:], AF.Exp)
                nc.scalar.activation(blockw[T:128, T:128], transt_sb[:],
                                     AF.Exp)

                ones64 = consts.tile([T, 1], F32, tag="ones64")
                nc.vector.memset(ones64[:], 1.0)
                negc0 = consts.tile([128, 1], F32, tag="negc0")
                nc.vector.memset(negc0[:], -C0)

                racc = consts.tile([2, n_ren * bsh], BF16, tag="racc")
                asum = consts.tile([128, n_chunks * num_slabs], F32,
                                   tag="asum")
                tsum = consts.tile([T, 1], F32, tag="tsum")
                nc.gpsimd.memset(racc[:], 1.0)
                nc.gpsimd.memset(asum[:], 0.0)
                nc.gpsimd.memset(tsum[:], 0.0)

                # numerator: trans part (one fused dot with host counts)
                def tsum_quantum():
                    scr3 = numscr.tile([T, T], F32, tag="nscr32")
                    nv.scalar_tensor_tensor(
                        scr3[:], cnt_sb[:], 1.0, trans_sb[:],
                        op0=ALU.bypass, op1=ALU.mult, accum_out=tsum[:])

                emg = {}      # em-chunk g -> tile [128, bsh, T] bf16
                ohmap = {}    # em-chunk g -> last one-hot slab tile
                exraw = {}    # x-chunk c -> tile [128, 128, bsh] bf16

                def dma_chunk(d):
                    xr = xrawpool.tile([128, 128, bsh], BF16, tag="xr")
                    nc.sync.dma_start(xr[:], em_x_d.ap()[d])
                    exraw[d] = xr
                    for g in (d, n_chunks - 1 - d):
                        eg = empool.tile([128, bsh, T], BF16, tag="em")
                        nc.sync.dma_start(eg[:], em_m_d.ap()[g])
                        emg[g] = eg

                def num_quanta(g):
                    """Numerator emission-part quanta for em chunk g:
                    num_slabs x (one-hot, fused mul-accum) over b-slabs."""
                    qs = []
                    if no_num:
                        return qs
                    for k in range(num_slabs):
                        def q_oh(g=g, k=k, part=None):
                            if part is None or part == 0:
                                oh = numscr.tile([128, slab_b * T], BF16,
                                                 tag="oh")
                                ohmap[g] = oh
                            else:
                                oh = ohmap[g]
                            if num_pool_ts:
                                for bb in range(slab_b):
                                    col = g * bsh + k * slab_b + bb
                                    nc.gpsimd.tensor_scalar(
                                        oh[:, bb * T:(bb + 1) * T],
                                        iota_bt[:, 0:T],
                                        tagsf_sb[:, col:col + 1],
                                        None, op0=ALU.is_equal)
                            elif num_bcast:
                                if part is None:
                                    lo, hi = 0, slab_b
                                else:
                                    w = slab_b // oh_split
                                    lo, hi = part * w, (part + 1) * w
                                tag_b = (tags_sb[:, g * bsh + k * slab_b + lo:
                                                 g * bsh + k * slab_b + hi]
                                         .unsqueeze(2)
                                         .broadcast_to([128, hi - lo, T]))
                                nv.tensor_tensor(
                                    oh[:, lo * T:hi * T],
                                    iota_bt[:, 0:(hi - lo) * T],
                                    tag_b, op=ALU.is_equal)
                            else:
                                for bb in range(slab_b):
                                    nv.tensor_scalar(
                                        oh[:, bb * T:(bb + 1) * T],
                                        iota_bt[:, 0:T],
                                        tags_sb[:, g * bsh + k * slab_b + bb:
                                                g * bsh + k * slab_b + bb + 1],
                                        None, op0=ALU.is_equal)
                        def q_acc(g=g, k=k, fin=(k == num_slabs - 1)):
                            scr = numscr.tile([128, slab_b * T], BF16,
                                              tag="nscr")
                            col = g * num_slabs + k
                            nv.scalar_tensor_tensor(
                                scr[:], ohmap[g][:], 1.0,
                                emg[g][:, k * slab_b:(k + 1) * slab_b, :],
                                op0=ALU.bypass, op1=ALU.mult,
                                accum_out=asum[:, col:col + 1])
                            if fin:
                                del emg[g]
                                del ohmap[g]
                        if num_bcast and not num_pool_ts and oh_split > 1:
                            for part in range(oh_split):
                                qs.append(
                                    lambda g=g, k=k, part=part:
                                    q_oh(g=g, k=k, part=part))
                        else:
                            qs.append(q_oh)
                        qs.append(q_acc)
                    return qs

                def x_quanta(c):
                    """ACT-exp quanta producing X chunk c from em_x."""
                    xc = xpool.tile([128, 128, bsh], F32, tag="xc")
                    if fake_x:
                        def q():
                            nc.gpsimd.memset(xc[:], 0.0133)
                        return xc, [q]
                    qs = []
                    for hj in range(4):
                        def q(hj=hj):
                            sl = slice(hj * 32, (hj + 1) * 32)
                            nc.scalar.activation(
                                xc[:, sl, :], exraw[c][:, sl, :],
                                AF.Exp, bias=negc0[:])
                            if hj == 3:
                                del exraw[c]
                        qs.append(q)
                    return xc, qs

                # ---------------- main pipeline ----------------
                from collections import deque
                bg = deque()
                xchunks = {}
                if not fake_x:
                    dma_chunk(0)
                    if half > 1:
                        dma_chunk(1)
                    if half > 2:
                        dma_chunk(2)
                # prime only the first exp slab inline; the rest drain
                # through bg one quantum per round
                xc, qs = x_quanta(0)
                qs[0]()
                bg.extend(qs[1:])
                xchunks[0] = xc
                if half > 1:
                    xc, qs = x_quanta(1)
                    bg.extend(qs)
                    xchunks[1] = xc

                state = spool.tile([128, bsh], BF16, tag="st")
                nc.vector.tensor_copy(state[:], xchunks[0][:, 0, :])

                ren_set = set(ren_rounds)
                pending = {}   # round -> (xscr tile written, c, j)
                rbc_todo = {}  # round -> (p tile, ren index)
                ren_idx = 0

                for r in range(1, n_rounds + 1):
                    c, j = r >> 7, r & 127
                    if j == 1:
                        if not fake_x and c + 3 <= half - 1:
                            dma_chunk(c + 3)
                        if c + 2 <= half - 1:
                            xc, qs = x_quanta(c + 2)
                            xchunks[c + 2] = xc
                            bg.extend(qs)
                            xchunks.pop(c - 1, None)
                    elif j == 64:
                        if r == 64:
                            bg.append(tsum_quantum)
                        if not fake_x:
                            bg.extend(num_quanta(c))
                            bg.extend(num_quanta(n_chunks - 1 - c))
                    if bg:
                        bg.popleft()()

                    # delayed renorm: broadcast rhat (2 rounds after log)
                    if r in rbc_todo:
                        rm, k = rbc_todo.pop(r)
                        rbc = prbc.tile([128, bsh], F32, tag="rbc")
                        if ren_inplace:
                            nc.tensor.matmul(
                                rbc[:], sel128[:],
                                racc128[:, k * bsh:(k + 1) * bsh],
                                start=True, stop=True)
                        else:
                            nc.tensor.matmul(
                                rbc[:], blocksel[:],
                                racc[:, k * bsh:(k + 1) * bsh],
                                start=True, stop=True)
                        pending[rm] = rbc

                    xsrc = xchunks[c][:, j, :]
                    if r in pending:
                        rbc = pending.pop(r)
                        xscr = smallpool.tile([128, bsh], F32, tag="xs")
                        ew.tensor_mul(xscr[:], xsrc, rbc[:])
                        xsrc = xscr[:]

                    p = pround.tile([128, bsh], F32, tag="p")
                    nc.tensor.matmul(p[:], blockw[:], state[:],
                                     start=True, stop=True)
                    if pe_warm == 1:
                        pd = pdummy.tile([128, 32], F32, tag="pd")
                        nc.tensor.matmul(pd[:, 0:1], blockw[:],
                                         blockw[:, 0:1],
                                         start=True, stop=True)
                        nc.tensor.matmul(pd[:, 0:1], blockw[:],
                                         blockw[:, 0:1],
                                         start=True, stop=True)
                    elif pe_warm == 2:
                        pd = pdummy.tile([128, 192], F32, tag="pd")
                        nc.tensor.matmul(pd[:], blockw[:],
                                         iota_bt[:, 0:192],
                                         start=True, stop=True)
                    state = spool.tile([128, bsh], BF16, tag="st")
                    ew.tensor_mul(state[:], p[:], xsrc)

                    if r in ren_set:
                        k = ren_idx
                        ren_idx += 1
                        if ren_inplace:
                            # partition-aligned: rows 0/64 of p are mass
                            # proxies; reciprocals land in the same rows
                            cols = slice(k * bsh, (k + 1) * bsh)
                            nc.vector.reciprocal(
                                racc128[0:1, cols], p[0:1, :])
                            nc.vector.reciprocal(
                                racc128[64:65, cols], p[64:65, :])
                        else:
                            # per-direction mass onto partitions 0:2, then
                            # log its bf16 reciprocal into the racc ring
                            mass = pmass.tile([2, bsh], F32, tag="mass")
                            nc.tensor.matmul(mass[:], blockones[:],
                                             state[:], start=True,
                                             stop=True)
                            nc.vector.reciprocal(
                                racc[:, k * bsh:(k + 1) * bsh], mass[:])
                        rbc_todo[r + 2] = (r + DELTA, k)

                while bg:
                    bg.popleft()()

                # ---------------- final combine ----------------
                # beta = W @ C on partitions 0..63 (aligned base-64 matmul)
                pf = pround.tile([128, bsh], F32, tag="p")
                nc.tensor.matmul(pf[0:T, :], blockw[T:128, T:128],
                                 state[T:128, :], start=True, stop=True)
                y = smallpool.tile([T, bsh], F32, tag="y")
                nc.vector.tensor_mul(y[:], state[0:T, :], pf[0:T, :])
                z = prbc.tile([128, bsh], F32, tag="rbc")
                nc.tensor.matmul(z[0:1, :], ones64[:], y[:],
                                 start=True, stop=True)
                z_sb = smallpool.tile([1, bsh], F32, tag="zsb")
                nc.vector.tensor_copy(z_sb[:], z[0:1, :])
                nc.sync.dma_start(z_d.ap(), z_sb[:])
                if ren_inplace:
                    nc.sync.dma_start(racc_d.ap()[0:1, :], racc128[0:1, :])
                    nc.sync.dma_start(racc_d.ap()[1:2, :],
                                      racc128[64:65, :])
                else:
                    nc.sync.dma_start(racc_d.ap(), racc[:])
                nc.sync.dma_start(asum_d.ap(), asum[:])
                nc.sync.dma_start(tsum_d.ap(), tsum[:])

    nc.compile()
    return nc


def _get_nc(n_chunks=16, bsh=BSH):
    key = (n_chunks, bsh)
    if key not in _NC_CACHE:
        _NC_CACHE[key] = build(n_chunks, bsh)
    return _NC_CACHE[key]


def _consts(n_chunks=16, bsh=BSH):
    # iota_bt[s, b*T + t] = t
    iota = np.broadcast_to(np.arange(T, dtype=F32_NP)[None, None, :],
                           (128, bsh, T)).reshape(128, bsh * T)
    iota = np.ascontiguousarray(iota).astype(BF16_NP)
    bsel = np.zeros((2, 128), dtype=F32_NP)
    bsel[0, 0:T] = 1.0
    bsel[1, T:128] = 1.0
    bones = np.zeros((128, 2), dtype=F32_NP)
    bones[0:T, 0] = 1.0
    bones[T:128, 1] = 1.0
    sel = np.zeros((128, 128), dtype=F32_NP)
    sel[0, 0:T] = 1.0
    sel[T, T:128] = 1.0
    return (iota, bsel.astype(BF16_NP), bones.astype(BF16_NP),
            sel.astype(BF16_NP))


def make_in_maps(emissions, start_transitions, end_transitions, transitions,
                 tags, ncores=NCORES):
    """Host prep: fold start/end into em, convert to bf16, build the two
    DMA-friendly layouts (em_x for the recurrence, em_m for the
    numerator), pair-count matrix from tags, shard over cores."""
    em = np.asarray(emissions, dtype=F32_NP).copy()
    em[:, 0, :] += np.asarray(start_transitions, dtype=F32_NP)
    em[:, -1, :] += np.asarray(end_transitions, dtype=F32_NP)
    em_b = em.astype(BF16_NP)
    b_all, s_len = em.shape[0], em.shape[1]
    n_chunks = s_len // CHUNK
    half = n_chunks // 2
    # em_x[c, row, j, b]: rows 0:64 fwd t of chunk c (s = 128c + j);
    # rows 64:128 bwd t of chunk n_chunks-1-c with j reversed
    # (s = s_len-1 - 128c - j)
    fwd = em_b[:, :half * 128, :].reshape(b_all, half, 128, T)
    fwd = fwd.transpose(1, 3, 2, 0)                    # [c, t, j, b]
    bwd = em_b[:, half * 128:, :].reshape(b_all, half, 128, T)
    bwd = bwd[:, ::-1, ::-1, :].transpose(1, 3, 2, 0)  # [c, t, j, b]
    em_x = np.concatenate([fwd, bwd], axis=1)          # [c, 128, 128, b]
    # em_m[g, s, b, t]
    em_m = em_b.reshape(b_all, n_chunks, 128, T).transpose(1, 2, 0, 3)
    tags_i = np.asarray(tags).astype(np.int64).reshape(b_all, s_len)
    # tags_t[s, g, b] = tags[b, 128g + s]  (bf16; 0..63 exact)
    tags_t = (tags_i.reshape(b_all, n_chunks, CHUNK)
              .transpose(2, 1, 0).astype(F32_NP))
    trans = np.asarray(transitions, dtype=F32_NP).reshape(T, T)
    iota, bsel, bones, sel128 = _consts(n_chunks)
    bsh = b_all // ncores
    in_maps = []
    for cidx in range(ncores):
        sl = slice(cidx * bsh, (cidx + 1) * bsh)
        # pair counts from this core's tags (index data only)
        pair = (T * tags_i[sl, :-1] + tags_i[sl, 1:]).ravel()
        cnt = np.bincount(pair, minlength=T * T).astype(F32_NP)
        in_maps.append({
            "emx": np.ascontiguousarray(em_x[:, :, :, sl]),
            "emm": np.ascontiguousarray(em_m[:, :, sl, :]),
            "tagst": np.ascontiguousarray(
                tags_t[:, :, sl].reshape(CHUNK, n_chunks * bsh)
            ).astype(BF16_NP),
            "tagsf": np.ascontiguousarray(
                tags_t[:, :, sl].reshape(CHUNK, n_chunks * bsh)),
            "cnt": cnt.reshape(T, T),
            "trans": trans,
            "transt": np.ascontiguousarray(trans.T),
            "bsel": bsel,
            "bones": bones,
            "sel128": sel128,
            "iotat": iota,
        })
    return in_maps


def kernel(emissions, start_transitions, end_transitions, transitions,
           tags, mask):
    """Full-input entry point; shards over 8 NeuronCores internally."""
    from concourse.bass_utils import run_bass_kernel_spmd

    emissions = np.asarray(emissions)
    assert emissions.shape == (B, S, T)
    assert (np.asarray(mask) != 0).all(), "kernel assumes all-ones mask"

    in_maps = make_in_maps(emissions, start_transitions, end_transitions,
                           transitions, tags)
    nc = _get_nc()
    res = run_bass_kernel_spmd(nc, in_maps, core_ids=list(range(NCORES)))

    num_total = 0.0
    den_total = 0.0
    for cidx in range(NCORES):
        r = res.results[cidx]
        num_total += float(r["asum"].sum()) + float(r["tsum"].sum())
        den = (np.log(r["z"].astype(np.float64))[0]
               + S * C0
               - np.log(r["racc"].astype(np.float64)
                        .reshape(2, -1, BSH)).sum(axis=(0, 1)))
        den_total += float(den.sum())
    loss = -(num_total - den_total) / float(B)
    return np.float32(loss)

